# revision 15
# baseline (speedup 1.0000x reference)
"""Trainium2 Bass kernel for nn_CrossChannelAttention.

Reference computation (per batch b, pixel p, with C=128 channels, NUMS=16
groups of HEADS=8 channels, OUT=256):
    fm[g,p]  = relu(sum_h W1[g,h] * x[8g+h, p] + b1[g])          # [16, P]
    feat[(g,d), p] = fm[g,p] * x[d,p]                            # [2048, P]
    out[o,p] = sum_c W2[o,c] * feat[c,p] + b2[o]                 # [256, P]

Strategy: data-parallel over batch B=8 across the 8 NeuronCores (one batch
image per core, params replicated).  Per core the kernel is PE-bound:
256 accumulating K=128/N=512 bf16 matmuls = ~55us of mandatory PE streaming.
Everything else is scheduled to hide under that stream:
  - inputs: x chunks split across the sync+gpsimd DMA queues, w2t/b2 on the
    vector queue (idle early), so the first fm matmul issues ~1us after boot
    instead of waiting on one serial queue.
  - fm rows are broadcast to 128 partitions via DRAM->SBUF broadcast DMAs
    only (round-robin over sync/gpsimd/scalar queues).  gpsimd
    partition_broadcast is NOT used: concurrent gpsimd SBUF writes were
    measured to stall DVE tensor_tensor reads (679ns -> 2087ns).
  - feat = x * fm_rep on the vector engine as pure-SBUF bf16 multiplies
    (2x mode, ~679ns per [128,1024]), LOOKAHEAD tiles ahead of the mains.
  - output bias+copy on the scalar engine, spread per pixel-block; the last
    block splits oc0->scalar / oc1->vector to shorten the tail; output DMAs
    alternate sync/gpsimd.
Accuracy: bf16 matmuls with fp32 PSUM accumulation; rel err ~4e-3.
"""

import numpy as np
import ml_dtypes

import concourse.bacc as bacc
import concourse.tile as tile
from concourse import mybir
from concourse.bass_utils import run_bass_kernel_spmd

F32 = mybir.dt.float32
BF16 = mybir.dt.bfloat16

B, C, H, W = 8, 128, 64, 64
NUMS, HEADS, OUT = 16, 8, 256
P = H * W          # 4096 pixels per image
PB = 512           # pixel block (one PSUM bank of fp32)
NPB = P // PB      # 8 pixel blocks
GRP = 1024         # broadcast chunk (2 pixel blocks)
NGRP = P // GRP    # 4 broadcast groups
N_CORES = 8
LOOKAHEAD = 16     # broadcast/feat pipeline depth (in (g,k) units) ahead of mains

_CACHE = {}


def _build():
    nc = bacc.Bacc("TRN2", target_bir_lowering=False, debug=False,
                   num_devices=N_CORES)

    x_d = nc.dram_tensor("x", [C, P], BF16, kind="ExternalInput")
    w1s_d = nc.dram_tensor("w1s", [C, NUMS], BF16, kind="ExternalInput")
    w2t_d = nc.dram_tensor("w2t", [C, NUMS * OUT], BF16, kind="ExternalInput")
    b1_d = nc.dram_tensor("b1c", [NUMS, 1], F32, kind="ExternalInput")
    b2_d = nc.dram_tensor("b2c", [C, 2], F32, kind="ExternalInput")
    out_d = nc.dram_tensor("out", [OUT, P], BF16, kind="ExternalOutput")

    relu = mybir.ActivationFunctionType.Relu
    ident = mybir.ActivationFunctionType.Identity
    mult = mybir.AluOpType.mult

    with tile.TileContext(nc) as tc:
        with (
            tc.tile_pool(name="const", bufs=1) as cpool,
            tc.tile_pool(name="xbp", bufs=1) as xbp,
            tc.tile_pool(name="repp", bufs=LOOKAHEAD + 2) as repp,
            tc.tile_pool(name="feat", bufs=LOOKAHEAD + 2) as featp,
            tc.tile_pool(name="osb", bufs=4) as osb,
            tc.tile_pool(name="psf", bufs=2, space="PSUM") as psf,
            tc.tile_pool(name="ps", bufs=6, space="PSUM") as ps,
            tc.tile_pool(name="dr", bufs=4, space="DRAM") as drp,
        ):
            # ---- inputs.  The sync+gpsimd DMA rings carry x early (the
            # ramp-critical transfer); the tiny-packet w1s/b1 go on the
            # scalar ring (tiny strided DMAs at the head of the x ring were
            # measured to stall it ~6us); w2t rides gpsimd interleaved so
            # chunk j arrives just before the mains reach its groups. ----
            x2s = []
            for k in range(NGRP):
                x2 = xbp.tile([C, GRP], BF16, tag=f"x2_{k}", name=f"x2_{k}")
                x2s.append(x2)
            w2t_t = cpool.tile([C, NUMS * OUT], BF16)
            # chunk 0 split across rings: x[:,0:512] gates the whole ramp,
            # x[:,512:1024] rides gpsimd in parallel
            nc.sync.dma_start(x2s[0][:, 0:PB], x_d[:, 0:PB])
            nc.gpsimd.dma_start(x2s[0][:, PB:GRP], x_d[:, PB:GRP])
            nc.sync.dma_start(x2s[2][:], x_d[:, 2 * GRP:3 * GRP])
            nc.gpsimd.dma_start(w2t_t[:, 0:GRP], w2t_d[:, 0:GRP])
            nc.gpsimd.dma_start(x2s[1][:], x_d[:, GRP:2 * GRP])
            nc.gpsimd.dma_start(w2t_t[:, GRP:2 * GRP], w2t_d[:, GRP:2 * GRP])
            nc.gpsimd.dma_start(x2s[3][:], x_d[:, 3 * GRP:4 * GRP])
            nc.gpsimd.dma_start(w2t_t[:, 2 * GRP:3 * GRP],
                                w2t_d[:, 2 * GRP:3 * GRP])
            nc.gpsimd.dma_start(w2t_t[:, 3 * GRP:4 * GRP],
                                w2t_d[:, 3 * GRP:4 * GRP])

            w1s_t = cpool.tile([C, NUMS], BF16)
            nc.scalar.dma_start(w1s_t[:], w1s_d[:])
            b1_t = cpool.tile([NUMS, 1], F32)
            nc.scalar.dma_start(b1_t[:], b1_d[:])
            b2_t = cpool.tile([C, 2], F32)
            nc.gpsimd.dma_start(b2_t[:], b2_d[:])

            # ---- fm machinery: per 512-pixel block, matmul + relu, then
            # DRAM round-trip (DMA partition-broadcast needs a DRAM source:
            # SBUF APs require nonzero partition step).  Replication via
            # broadcast DMAs; feat on the vector engine. ----
            fm_sb = cpool.tile([NUMS, P], BF16)
            fm_drs = [drp.tile([NUMS, GRP], BF16, tag=f"fmdr{k}",
                               name=f"fmdr{k}")
                      for k in range(NGRP)]

            def emit_fm(k):
                for half in range(2):
                    pb = 2 * k + half
                    px = slice(pb * PB, (pb + 1) * PB)
                    hx = slice(half * PB, (half + 1) * PB)
                    ps_fm = psf.tile([NUMS, PB], F32, tag="psf",
                                     name=f"psfm{pb}")
                    nc.tensor.matmul(ps_fm[:], w1s_t[:], x2s[k][:, hx],
                                     start=True, stop=True)
                    nc.scalar.activation(fm_sb[:, px], ps_fm[:], relu,
                                         bias=b1_t[:])

            nbc = [0]
            BCAST_ENGS = (nc.sync, nc.scalar, nc.gpsimd)
            fts = {}      # (g, k) -> [C, GRP] feat tile

            def emit_ft(g, k, eng=None):
                rep = repp.tile([C, GRP], BF16, tag="rep", name=f"rep{g}_{k}")
                if eng is None:
                    eng = BCAST_ENGS[nbc[0] % 3]
                    nbc[0] += 1
                eng.dma_start(rep[:],
                              fm_drs[k][g:g + 1, :].broadcast_to((C, GRP)))
                ft = featp.tile([C, GRP], BF16, tag="ft", name=f"ft{g}_{k}")
                nc.vector.tensor_tensor(ft[:], x2s[k][:], rep[:], op=mult)
                fts[(g, k)] = ft

            # ---- ramp: interleave fm chunks with the k=0 pipeline so the
            # first feat tiles are ready as soon as possible after the k=0
            # activations. ----
            emit_fm(0)
            # fm chunk 0 -> DRAM in two halves (each behind one ACT only)
            for half in range(2):
                hx = slice(half * PB, (half + 1) * PB)
                nc.sync.dma_start(fm_drs[0][:, hx], fm_sb[:, hx])
            for g in range(0, 6):
                emit_ft(g, 0, eng=(nc.scalar if g % 2 else nc.sync))
            emit_fm(1)
            nc.sync.dma_start(fm_drs[1][:], fm_sb[:, GRP:2 * GRP])
            for g in range(6, 10):
                emit_ft(g, 0, eng=(nc.scalar if g % 2 else nc.sync))
            emit_fm(2)
            nc.sync.dma_start(fm_drs[2][:], fm_sb[:, 2 * GRP:3 * GRP])
            for g in range(10, 13):
                emit_ft(g, 0, eng=(nc.scalar if g % 2 else nc.sync))
            emit_fm(3)
            nc.sync.dma_start(fm_drs[3][:], fm_sb[:, 3 * GRP:4 * GRP])
            for g in range(13, NUMS):
                emit_ft(g, 0, eng=(nc.scalar if g % 2 else nc.sync))

            todo = [(g, k) for k in range(NGRP) for g in range(NUMS)]
            nemit = [NUMS]    # k=0 fully emitted above

            nout = [0]
            pso = {}
            for i, (g, k) in enumerate(todo):
                while nemit[0] < len(todo) and nemit[0] < i + LOOKAHEAD + 1:
                    emit_ft(*todo[nemit[0]])
                    nemit[0] += 1
                ft = fts.pop((g, k))
                if g == 0:
                    for pbb in (2 * k, 2 * k + 1):
                        for oc in range(2):
                            t = ps.tile([C, PB], F32, tag="ps",
                                        name=f"pso{pbb}_{oc}")
                            pso[(pbb, oc)] = t
                for half in range(2):
                    pb = 2 * k + half
                    hx = slice(half * PB, (half + 1) * PB)
                    nc.tensor.matmul(pso[(pb, 0)][:],
                                     w2t_t[:, (2 * g) * C:(2 * g + 1) * C],
                                     ft[:, hx], start=(g == 0),
                                     stop=(g == NUMS - 1))
                    nc.tensor.matmul(pso[(pb, 1)][:],
                                     w2t_t[:, (2 * g + 1) * C:(2 * g + 2) * C],
                                     ft[:, hx], start=(g == 0),
                                     stop=(g == NUMS - 1))
                if g == NUMS - 1:
                    last_k = (k == NGRP - 1)
                    for pbb in (2 * k, 2 * k + 1):
                        px = slice(pbb * PB, (pbb + 1) * PB)
                        o0 = osb.tile([C, PB], BF16, tag="osb",
                                      name=f"o0_{pbb}")
                        o1 = osb.tile([C, PB], BF16, tag="osb",
                                      name=f"o1_{pbb}")
                        nc.scalar.activation(o0[:], pso.pop((pbb, 0))[:],
                                             ident, bias=b2_t[:, 0:1])
                        if last_k:
                            # split the tail: oc1 bias-add on the vector
                            # engine (its TT stream is done by now)
                            nc.vector.tensor_scalar_add(
                                o1[:], pso.pop((pbb, 1))[:], b2_t[:, 1:2])
                        else:
                            nc.scalar.activation(o1[:], pso.pop((pbb, 1))[:],
                                                 ident, bias=b2_t[:, 1:2])
                        e0 = (nc.sync, nc.gpsimd)[nout[0] % 2]
                        nout[0] += 1
                        e1 = (nc.sync, nc.gpsimd)[nout[0] % 2]
                        nout[0] += 1
                        e0.dma_start(out_d[0:C, px], o0[:])
                        e1.dma_start(out_d[C:OUT, px], o1[:])

    nc.compile()
    return nc


def _prep_params(W1, b1, W2, b2):
    bf = ml_dtypes.bfloat16
    # w1s[c, g] = W1[g, c - 8g] for 8g <= c < 8(g+1), else 0
    w1s = np.zeros((C, NUMS), dtype=bf)
    for g in range(NUMS):
        w1s[g * HEADS:(g + 1) * HEADS, g] = W1[g].astype(bf)
    # w2t[k, (g*2+oc)*128 + m] = W2[oc*128 + m, g*128 + k]
    w2t = (
        np.asarray(W2, dtype=np.float32)
        .reshape(2, C, NUMS, C)          # [oc, m, g, k]
        .transpose(3, 2, 0, 1)           # [k, g, oc, m]
        .reshape(C, NUMS * OUT)
        .astype(bf)
    )
    b1c = np.asarray(b1, dtype=np.float32).reshape(NUMS, 1).copy()
    b2c = np.asarray(b2, dtype=np.float32).reshape(2, C).T.copy()
    return w1s, w2t, b1c, b2c


def kernel(x, W1, b1, W2, b2, _trace=False, _trace_kwargs=None):
    if "nc" not in _CACHE:
        _CACHE["nc"] = _build()
    nc = _CACHE["nc"]

    w1s, w2t, b1c, b2c = _prep_params(W1, b1, W2, b2)
    xs = np.ascontiguousarray(
        np.asarray(x, dtype=np.float32).reshape(B, C, P).astype(ml_dtypes.bfloat16))
    in_maps = [
        {"x": xs[b_], "w1s": w1s, "w2t": w2t, "b1c": b1c, "b2c": b2c}
        for b_ in range(N_CORES)
    ]
    kwargs = {}
    if _trace:
        kwargs["trace"] = True
        kwargs.update(_trace_kwargs or {})
    res = run_bass_kernel_spmd(nc, in_maps, core_ids=list(range(N_CORES)),
                               **kwargs)
    out = np.stack([np.asarray(res.results[b_]["out"], dtype=np.float32)
                    for b_ in range(N_CORES)])
    out = out.reshape(B, OUT, H, W)
    if _trace:
        _CACHE["last_result"] = res
    return out
